# revision 8
# baseline (speedup 1.0000x reference)
"""Multi-head self-attention (B=4, S=2048, D=1024, 16 heads x 64) on 8 TRN2
NeuronCores via Bass/Tile.

Sharding: tensor-parallel over heads. Each core owns 2 heads (128 of the 1024
Q/K/V output features, column-parallel) and the matching 128 rows of Wo
(row-parallel). Every core computes a full-shape partial output; the host sums
the 8 partials (the row-parallel all-reduce) after gathering.

Per-core dataflow (matmul operands bf16, accumulation fp32 in PSUM):
  xT[b]   : [D, S] features-on-partitions (host pre-transposed)
  qT/kT/vT: [128, S]  = (x @ W)^T per core, via lhsT=W k-tiles, rhs=xT
  v_aug   : PE-transpose of vT -> v natural [S,64] per head + ones column
  scoresT : [j, i] per j-tile; the two heads run as concurrent row-groups
            (0-63 / 64-127) of the PE array into one [128,1024] PSUM tile
  exp     : one ACT Exp op per (i-chunk, j-tile) covering both heads
            (scale=1/8, per-partition bias = attention-mask column)
  PV      : lhsT=[v_h | ones] [128 j, 65], rhs=expT half, accumulated over
            j-tiles -> rows 0-63 ctx^T, row 64 = softmax denominator
  norm    : reciprocal of denom row + log-doubling partition broadcast on DVE,
            multiply -> ctxT [128, S] bf16
  out     : lhsT=ctxT tile [128,128], rhs=Wo_c [128,512] chunks; DVE adds
            bo/8 while copying PSUM->SBUF; DMA partial to DRAM

The emission is software-pipelined: batch b's attention steps are interleaved
with batch b+1's QKV/V-transpose units and batch b-1's output-projection
units, so the (in-order) PE always has independent matmul work while the
ACT-bound softmax stream runs, keeping the PE HAM clock-gate warm.
"""

import numpy as np
import ml_dtypes

import concourse.bass as bass
import concourse.mybir as mybir
import concourse.tile as tile
from concourse import bacc, bass_utils
from concourse.masks import make_identity

F32 = mybir.dt.float32
BF16 = mybir.dt.bfloat16
AF = mybir.ActivationFunctionType
BF = ml_dtypes.bfloat16
ts = bass.ts

B, S, D = 4, 2048, 1024
NH, HD = 16, 64
NCORES = 8
OF = D // NCORES            # 128 out-features per core (2 heads)
NKT = D // 128              # 8 contraction tiles
NJT = S // 128              # 16 key tiles per batch
NICH = S // 512             # 4 query chunks per batch
NTT = S // 128              # 16 token tiles per batch


def build_program():
    nc = bacc.Bacc("TRN2", target_bir_lowering=False, debug=False,
                   num_devices=NCORES)
    xT_d = nc.dram_tensor("xT", [B, D, S], BF16, kind="ExternalInput")
    wq_d = nc.dram_tensor("wq", [D, OF], BF16, kind="ExternalInput")
    wk_d = nc.dram_tensor("wk", [D, OF], BF16, kind="ExternalInput")
    wv_d = nc.dram_tensor("wv", [D, OF], BF16, kind="ExternalInput")
    bq_d = nc.dram_tensor("bq", [OF, 1], F32, kind="ExternalInput")
    bk_d = nc.dram_tensor("bk", [OF, 1], F32, kind="ExternalInput")
    bv_d = nc.dram_tensor("bv", [OF, 1], F32, kind="ExternalInput")
    wo_d = nc.dram_tensor("wo", [OF, D], BF16, kind="ExternalInput")
    bo8_d = nc.dram_tensor("bo8", [128, D], F32, kind="ExternalInput")
    mask_d = nc.dram_tensor("mask", [128, B * NJT], F32, kind="ExternalInput")
    out_d = nc.dram_tensor("out", [B * S, D], F32, kind="ExternalOutput")

    with tile.TileContext(nc) as tc:
        with (
            tc.tile_pool(name="consts", bufs=1) as consts,
            tc.tile_pool(name="xin", bufs=3) as xin,
            tc.tile_pool(name="qkv", bufs=2) as qkv,
            tc.tile_pool(name="attn", bufs=4) as attn,
            tc.tile_pool(name="ctxp", bufs=2) as ctxp,
            tc.tile_pool(name="outp", bufs=3) as outp,
            tc.tile_pool(name="psum", bufs=2, space="PSUM") as psum,
        ):
            # ---------------- constants ----------------
            ident = consts.tile([128, 128], BF16)
            make_identity(nc, ident)
            w_sb = {}
            for nm, d in (("q", wq_d), ("k", wk_d), ("v", wv_d)):
                t = consts.tile([128, NKT, OF], BF16, name=f"w{nm}_sb")
                nc.sync.dma_start(t, d[:, :].rearrange("(k p) f -> p k f", p=128))
                w_sb[nm] = t
            b_sb = {}
            for nm, d in (("q", bq_d), ("k", bk_d), ("v", bv_d)):
                t = consts.tile([OF, 1], F32, name=f"b{nm}_sb")
                nc.sync.dma_start(t, d[:, :])
                b_sb[nm] = t
            wo_sb = consts.tile([OF, D], BF16)
            nc.sync.dma_start(wo_sb, wo_d[:, :])
            bo_sb = consts.tile([128, D], F32)
            nc.sync.dma_start(bo_sb, bo8_d[:, :])
            mask_sb = consts.tile([128, B * NJT], F32)
            nc.sync.dma_start(mask_sb, mask_d[:, :])

            state = [dict() for _ in range(B)]

            def qkv_units(b):
                """QKV projections + V transpose for batch b (chunk-paired
                so accumulation chains alternate PSUM banks)."""
                st = state[b]
                pT = {nm: qkv.tile([OF, S], BF16, name=f"{nm}T")
                      for nm in ("q", "k", "v")}
                st["pT"] = pT
                va = [qkv.tile([128, NJT, 65], BF16, name=f"v_aug{h}")
                      for h in range(2)]
                st["va"] = va
                for h in range(2):
                    nc.vector.memset(va[h][:, :, 64:65], 1.0)
                for nch in range(NICH):
                    xt = xin.tile([128, NKT, 512], BF16, name="xt")
                    nc.sync.dma_start(
                        xt,
                        xT_d[b].rearrange("(k p) t -> p k t", p=128)[
                            :, :, ts(nch, 512)],
                    )
                    yield
                    for nm in ("q", "k", "v"):
                        ps = psum.tile([128, 512], F32, tag="mm",
                                       name="ps_qkv")
                        for kt in range(NKT):
                            nc.tensor.matmul(
                                ps, lhsT=w_sb[nm][:, kt, :],
                                rhs=xt[:, kt, :],
                                start=(kt == 0), stop=(kt == NKT - 1),
                            )
                            if kt % 2 == 1:
                                yield
                        nc.vector.tensor_scalar_add(
                            pT[nm][:, ts(nch, 512)], ps, b_sb[nm])
                    for jt in range(4 * nch, 4 * nch + 4):
                        for h in range(2):
                            hs = slice(h * 64, (h + 1) * 64)
                            pvt = psum.tile([128, 64], BF16, tag="mm",
                                            name="pvt")
                            nc.tensor.transpose(
                                pvt, pT["v"][hs, ts(jt, 128)], ident[hs, hs])
                            nc.vector.tensor_copy(va[h][:, jt, 0:64], pvt)
                        yield

            def attn_units(b):
                """Attention for batch b. 68 yields."""
                st = state[b]
                qT, kT = st["pT"]["q"], st["pT"]["k"]
                va = st["va"]
                ctxT = ctxp.tile([128, S], BF16, name="ctxT")
                st["ctxT"] = ctxT
                for ich in range(NICH):
                    isl = ts(ich, 512)
                    pc = [psum.tile([128, 512], F32, tag="pc", name=f"pc{h}")
                          for h in range(2)]
                    for jt in range(NJT):
                        sc = psum.tile([128, 1024], F32, tag="sc", name="sc")
                        for h in range(2):
                            hs = slice(h * 64, (h + 1) * 64)
                            nc.tensor.matmul(
                                sc[:, ts(h, 512)],
                                lhsT=kT[hs, ts(jt, 128)], rhs=qT[hs, isl],
                                start=True, stop=True,
                            )
                        et = attn.tile([128, 1024], BF16, name="et", bufs=6)
                        col = b * NJT + jt
                        nc.scalar.activation(
                            et, sc, AF.Exp,
                            bias=mask_sb[:, col:col + 1], scale=0.125)
                        for h in range(2):
                            nc.tensor.matmul(
                                pc[h][0:65, :], lhsT=va[h][:, jt, :],
                                rhs=et[:, ts(h, 512)],
                                start=(jt == 0), stop=(jt == NJT - 1),
                            )
                        yield
                    for h in range(2):
                        den = attn.tile([1, 512], F32, name=f"den{h}")
                        nc.vector.tensor_copy(den, pc[h][64:65, :])
                        rec = attn.tile([1, 512], F32, name=f"rec{h}")
                        nc.vector.reciprocal_approx_fast(rec, den)
                        rep = attn.tile([64, 512], F32, name=f"rep{h}")
                        nc.gpsimd.partition_broadcast(rep, rec)
                        nc.vector.tensor_mul(
                            ctxT[h * 64:(h + 1) * 64, isl],
                            pc[h][0:64, :], rep)
                        yield

            def outproj_units(b):
                """Output projection for batch b. 32 yields."""
                ctxT = state[b]["ctxT"]
                for tt in range(NTT):
                    for oc in range(2):
                        po = psum.tile([128, 512], F32, tag="mm", name="po")
                        nc.tensor.matmul(
                            po, lhsT=ctxT[:, ts(tt, 128)],
                            rhs=wo_sb[:, ts(oc, 512)],
                            start=True, stop=True,
                        )
                        osb = outp.tile([128, 512], F32, name="osb")
                        nc.vector.tensor_add(osb, po, bo_sb[:, ts(oc, 512)])
                        nc.sync.dma_start(
                            out_d[b * S + tt * 128: b * S + (tt + 1) * 128,
                                  ts(oc, 512)],
                            osb)
                        yield

            def drain(*weighted):
                """weighted: (gen, stride) — advance gen every `stride`
                cycles. Round-robin until all exhausted."""
                live = [(g, s) for g, s in weighted if g is not None]
                cyc = 0
                while live:
                    nxt = []
                    for g, s in live:
                        if cyc % s == 0:
                            try:
                                next(g)
                            except StopIteration:
                                continue
                        nxt.append((g, s))
                    live = nxt
                    cyc += 1

            def pull(g, n):
                for _ in range(n):
                    try:
                        next(g)
                    except StopIteration:
                        return False
                return True

            g_attn = [attn_units(b) for b in range(B)]
            g_qkv = [qkv_units(b) for b in range(B)]
            g_out = [outproj_units(b) for b in range(B)]

            # prologue: batch 0 QKV chunk 0 first, then pace attention(0)
            # in at 1:4 while the rest of QKV(0) streams
            pull(g_qkv[0], 17)
            drain((g_attn[0], 4), (g_qkv[0], 1))
            for b in range(B):
                drain(
                    (g_attn[b], 1),
                    (g_qkv[b + 1] if b + 1 < B else None, 1),
                    (g_out[b - 1] if b >= 1 else None, 2),
                )
            drain((g_out[B - 1], 1))
    nc.finalize()
    return nc


def make_in_maps(x, attention_mask, Wq, bq, Wk, bk, Wv, bv, Wo, bo):
    x = np.asarray(x, dtype=np.float32)
    attention_mask = np.asarray(attention_mask, dtype=np.float32)
    Wq, Wk, Wv, Wo = (np.asarray(a, dtype=np.float32) for a in (Wq, Wk, Wv, Wo))
    bq, bk, bv, bo = (np.asarray(a, dtype=np.float32) for a in (bq, bk, bv, bo))

    xT = np.ascontiguousarray(x.transpose(0, 2, 1)).astype(BF)  # [B, D, S]
    # mask[b,0,0,j] -> [128 partitions, B*NJT] column per (batch, j-tile)
    m = attention_mask.reshape(B, S).reshape(B, NJT, 128)
    mask_host = np.ascontiguousarray(m.transpose(2, 0, 1).reshape(128, B * NJT))
    bo8 = np.broadcast_to(bo / NCORES, (128, D)).astype(np.float32).copy()

    in_maps = []
    for c in range(NCORES):
        cs = slice(c * OF, (c + 1) * OF)
        in_maps.append({
            "xT": xT,
            "wq": np.ascontiguousarray(Wq[:, cs]).astype(BF),
            "wk": np.ascontiguousarray(Wk[:, cs]).astype(BF),
            "wv": np.ascontiguousarray(Wv[:, cs]).astype(BF),
            "bq": np.ascontiguousarray(bq[cs]).reshape(OF, 1),
            "bk": np.ascontiguousarray(bk[cs]).reshape(OF, 1),
            "bv": np.ascontiguousarray(bv[cs]).reshape(OF, 1),
            "wo": np.ascontiguousarray(Wo[cs, :]).astype(BF),
            "bo8": bo8,
            "mask": mask_host,
        })
    return in_maps


def combine_outputs(results):
    acc = np.zeros((B * S, D), dtype=np.float64)
    for r in results:
        acc += r["out"].astype(np.float64)
    return acc.reshape(B, S, D).astype(np.float32)


_NC_CACHE = []


def _get_program():
    if not _NC_CACHE:
        _NC_CACHE.append(build_program())
    return _NC_CACHE[0]


def kernel(**inputs):
    nc = _get_program()
    in_maps = make_in_maps(**inputs)
    res = bass_utils.run_bass_kernel_spmd(
        nc, in_maps, core_ids=list(range(NCORES)))
    return combine_outputs(res.results)
